# revision 15
# baseline (speedup 1.0000x reference)
"""Causal attention (B=4, S=2048, D=1024, single 1024-dim head) on 8 TRN2 cores.

Sharding: data-parallel over batch (4) x 2-way causal-balanced query split
(zigzag 256-row query blocks: core A gets global blocks {0,3,4,7}, core B
{1,2,5,6}).  SPMD: one program for all 8 cores; per-core differences are
expressed purely through data.

Key structure (v2):
  * Projections run QT-first:  QT = Wq x_q^T  [dout, 1024q], then
    GQ = Wk^T QT = (Wq^T Wk)^T x_q^T  [din, 1024q].  Same PE cycles as the
    MT-first form but the first chain needs only ~1MB of DMA (vs 4MB for
    MT = Wq^T Wk), so the PE starts real work as soon as the first
    xT column-block + wqT quarter land.
  * No separate gathered-xq input: each core's xT arrives with its columns
    PERMUTED so its own 4 query blocks sit at fixed positions {0,2,4,6}
    (perm A = [0,1,3,2,4,5,7,6], B = [1,0,2,3,5,4,6,7]).  QT reads the
    query columns straight out of the resident xT tile at uniform offsets;
    phase-2 key chunks walk the same permuted order, which keeps every
    slot's needed key blocks inside its chunk prefix with the usual
    "mask the last 4 chunks" structure (verified per slot/half).
    xn (the AV stationary operand) is row-permuted identically.
  * Few, large, strided DMAs (AP rearrange): one per xT 256-col block (8),
    one per wqT quarter (4), wk/masks/xn/wvT in 1-2 each -- ~15 input DMA
    instructions total (descriptor-gen on the sync queue costs ~650ns per
    instruction, so many small DMAs throttle delivery).
  * onesT / ones32 come from memsets (no DMA), so the PE warmup is gated
    only on the ~6us framework preamble, not on first-DMA latency.

Phase 2/3 (unchanged from v1): per q-slot (256 cols), per k-chunk (128 rows):
      sT   = xT_chunk^T GQ_slot [128k, 256q]  (PSUM, 8 din-chunk matmuls)
      expT = exp(sT/32)         (ACT, PSUM->SBUF fp16)
      mask-multiply (DVE) for the last 4 chunks of the slot (host tiles)
      dacc += expT              (DVE fp32 partial sums)
      ctxU[d] += xn_chunk[:,d]^T expT   (PSUM accumulate)
  deferred denominator finishers (dsum reduce + reciprocal, partition
  broadcast) drain inside later slots' chunk streams;
  outT = (WvT^T ctxU) * (1/denominator)   [dout, 1024]
"""

import os
import sys

sys.path.insert(0, "/opt/trn_rl_repo")

import numpy as np

B, S, DIN, DOUT = 4, 2048, 1024, 1024
P = 128
NQ = 1024  # q rows per core
ND = DIN // P
NO = DOUT // P
NK = S // P  # 16 key chunks
NCORES = 8
G = [[0, 3, 4, 7], [1, 2, 5, 6]]  # global 256-row q-block per (core-half, slot)
PERM = [[0, 1, 3, 2, 4, 5, 7, 6], [1, 0, 2, 3, 5, 4, 6, 7]]  # xT col-block order
L = [4, 8, 12, 16]  # k-chunks processed per slot (uniform across cores)

_NC_CACHE = {}


def _build_nc():
    import concourse.mybir as mybir
    import concourse.tile as tile
    from concourse import bacc
    from contextlib import ExitStack

    f32 = mybir.dt.float32
    f16 = mybir.dt.float16
    EXP = mybir.ActivationFunctionType.Exp

    nc = bacc.Bacc("TRN2", target_bir_lowering=False, debug=False,
                   num_devices=NCORES)

    # xT arrives host-packed as [blk, p, e, c] -> [128*blk + p, 256*e + c]:
    # one plain [128, 2048] row-copy per 256-col key block, 4KB DMA lines.
    # Partition p is the low 7 bits of din; e (din high bits) lives in the
    # free dim, so both the QT moving slices and the score stationary
    # slices are natural 2D sub-ranges.
    xT_d = nc.dram_tensor("xT", [8 * P, 8 * 256], f16, kind="ExternalInput").ap()
    xn_d = nc.dram_tensor("xn", [S, DIN], f16, kind="ExternalInput").ap()
    # wqT host-packed as [quarter, p, e, oc] -> [128*q + p, 256*e + oc]
    wqT_d = nc.dram_tensor("wqT", [4 * P, 8 * 256], f16, kind="ExternalInput").ap()
    wk_d = nc.dram_tensor("wk", [DOUT, DIN], f16, kind="ExternalInput").ap()
    wvT_d = nc.dram_tensor("wvT", [DIN, DOUT], f16, kind="ExternalInput").ap()
    masks_d = nc.dram_tensor("masks", [P, 16 * 256], f16, kind="ExternalInput").ap()
    outT_d = nc.dram_tensor("outT", [DOUT, NQ], f32, kind="ExternalOutput").ap()

    with tile.TileContext(nc) as tc:
        with ExitStack() as es:
            gq_pool = es.enter_context(tc.tile_pool(name="gqp", bufs=1))
            xt_pool = es.enter_context(tc.tile_pool(name="xtp", bufs=1))
            ctx_pool = es.enter_context(tc.tile_pool(name="ctxp", bufs=1))
            cst_pool = es.enter_context(tc.tile_pool(name="cst", bufs=1))
            xn_pool = es.enter_context(tc.tile_pool(name="xnp", bufs=1))
            wv_pool = es.enter_context(tc.tile_pool(name="wvp", bufs=1))
            bcs_pool = es.enter_context(tc.tile_pool(name="bcsp", bufs=1))

            # big flat tiles; logical chunk i lives at col i*width
            xTt = xt_pool.tile([P, ND * S], f16, name="xTt", tag="xTt")
            GQ = gq_pool.tile([P, ND * NQ], f16, name="GQ", tag="GQ")
            xn16 = xn_pool.tile([P, NK * DIN], f16, name="xn16", tag="xn16")
            wvs = wv_pool.tile([P, ND * DOUT], f16, name="wvs", tag="wvs")
            maskT = cst_pool.tile([P, 16 * 256], f16, name="maskT", tag="maskT")
            onesT = cst_pool.tile([P, 512], f16, name="onesT", tag="onesT")
            ones32 = cst_pool.tile([P, 1], f32, name="ones32", tag="ones32")
            nc.gpsimd.memset(onesT[:], 1.0)
            nc.gpsimd.memset(ones32[:], 1.0)
            ones_row = onesT[0:1, 0:128]  # [1, 128] of ones

            def xt_block(pos):
                nc.sync.dma_start(
                    xTt[:, 2048 * pos:2048 * (pos + 1)],
                    xT_d[P * pos:P * (pos + 1), :])

            def xt_at(dchunk, key):
                # column of the blocked xT tile holding (din-chunk, key)
                return 2048 * (key // 256) + 256 * dchunk + (key % 256)

            # -------- phase 1: QT = Wq xq^T, then GQ = Wk^T QT --------
            with tc.tile_pool(name="wqp", bufs=1) as wq_pool, \
                 tc.tile_pool(name="wkp", bufs=1) as wk_pool, \
                 tc.tile_pool(name="qtp", bufs=1) as qt_pool, \
                 tc.tile_pool(name="wups", bufs=1, space="PSUM") as wu_ps, \
                 tc.tile_pool(name="qps", bufs=3, space="PSUM") as qt_ps, \
                 tc.tile_pool(name="gps", bufs=3, space="PSUM") as gq_ps:
                wqT = wq_pool.tile([P, 4 * 2048], f16, name="wqT", tag="wqT")
                wk = wk_pool.tile([P, NO * DIN], f16, name="wk", tag="wk")
                QT = qt_pool.tile([P, NO * NQ], f16, name="QT", tag="QT")

                wkv = wk[:].rearrange("p (o c) -> p o c", o=NO)
                wk_sv = wk_d.rearrange("(o p) c -> p o c", p=P)

                def wq_quarter(q):
                    nc.sync.dma_start(wqT[:, 2048 * q:2048 * (q + 1)],
                                      wqT_d[P * q:P * (q + 1), :])

                # DMA order = need order
                xt_block(0)
                wq_quarter(0)
                wq_quarter(1)
                xt_block(2)
                wq_quarter(2)
                xt_block(4)
                wq_quarter(3)
                xt_block(6)
                nc.sync.dma_start(wkv[:], wk_sv[:])
                nc.sync.dma_start(maskT[:], masks_d[:])
                xt_block(1)
                xt_block(3)
                xnv = xn16[:].rearrange("p (k c) -> p k c", k=NK)
                xn_sv = xn_d.rearrange("(k p) c -> p k c", p=P)
                nc.sync.dma_start(xnv[:, 0:8, :], xn_sv[:, 0:8, :])
                xt_block(5)
                xt_block(7)
                nc.sync.dma_start(xnv[:, 8:16, :], xn_sv[:, 8:16, :])
                wvv = wvs[:].rearrange("p (e c) -> p e c", e=ND)
                wv_sv = wvT_d.rearrange("(e p) c -> p e c", p=P)
                nc.sync.dma_start(wvv[:], wv_sv[:])

                # PE warmup on the memset ones tile: covers the framework
                # preamble tail + first-DMA latency (~6-12us) so the HAM
                # clock gate is open when the first real chain lands.
                wu = wu_ps.tile([P, 512], f32, name="wu", tag="wu")
                for r in range(14):
                    nc.tensor.matmul(wu[:], onesT[:, 0:128], onesT[:],
                                     start=True, stop=True,
                                     skip_group_check=True)

                # 1a: QT[o, q] = sum_e wqT[e][:,o]^T xT[e][:, own-block qb]
                #     (own query blocks sit at permuted positions 2*qb)
                for qb in range(4):
                    for o in range(NO):
                        qp = qt_ps.tile([P, 256], f32, name="qp", tag="qp")
                        for e in range(ND):
                            wq_col = 2048 * (o // 2) + 256 * e + 128 * (o % 2)
                            nc.tensor.matmul(
                                qp[:],
                                wqT[:, wq_col:wq_col + 128],
                                xTt[:, 2048 * (2 * qb) + 256 * e:
                                    2048 * (2 * qb) + 256 * e + 256],
                                start=(e == 0), stop=(e == ND - 1))
                        dst = QT[:, 1024 * o + 256 * qb:1024 * o + 256 * qb + 256]
                        if o % 2 == 0:
                            nc.vector.tensor_copy(dst, qp[:])
                        else:
                            nc.scalar.copy(dst, qp[:])

                # 1b: GQ[d, q] = sum_o wk[o][:,d]^T QT[o]
                for h in range(2):
                    for d in range(ND):
                        gp = gq_ps.tile([P, 512], f32, name="gp", tag="gp")
                        for o in range(NO):
                            nc.tensor.matmul(
                                gp[:],
                                wk[:, 1024 * o + 128 * d:1024 * o + 128 * d + 128],
                                QT[:, 1024 * o + 512 * h:1024 * o + 512 * h + 512],
                                start=(o == 0), stop=(o == NO - 1))
                        dst = GQ[:, 1024 * d + 512 * h:1024 * d + 512 * h + 512]
                        if d % 2 == 0:
                            nc.vector.tensor_copy(dst, gp[:])
                        else:
                            nc.scalar.copy(dst, gp[:])

            # ------------- phase 2: attention (+ fused phase 3) -------------
            rec16 = {}
            # flat per-d ctx tiles [128, 1024q] and flat reciprocal tile so
            # phase 3 can run 512-wide (two slots per chain)
            ctxC = [ctx_pool.tile([P, NQ], f16, name=f"ctxc{d}", tag=f"ctxc{d}")
                    for d in range(ND)]
            bcsF = bcs_pool.tile([P, NQ], f32, name="bcsF", tag="bcsF")
            with tc.tile_pool(name="exq", bufs=5) as exp_pool, \
                 tc.tile_pool(name="dac", bufs=2) as dacc_pool, \
                 tc.tile_pool(name="obp", bufs=6) as out_pool, \
                 tc.tile_pool(name="sps", bufs=3, space="PSUM") as sT_ps, \
                 tc.tile_pool(name="cps", bufs=4, space="PSUM") as ctx_ps, \
                 tc.tile_pool(name="dps", bufs=1, space="PSUM") as dn_ps:

                # Deferred denominator finishers, drained one per hook point
                # in LATER slots' chunk streams / between phase-3 chains.
                def make_dsum(s, dacc):
                    def emit():
                        dsum = dn_ps.tile([1, 256], f32, name=f"dsum{s}",
                                          tag="dsum")
                        nc.tensor.matmul(dsum[:], ones32[:], dacc[:],
                                         start=True, stop=True)
                        r32 = cst_pool.tile([1, 256], f32, name=f"r32_{s}",
                                            tag=f"r32_{s}")
                        nc.vector.reciprocal_approx_fast(r32[:], dsum[:])
                        r16 = cst_pool.tile([1, 256], f16, name=f"rec{s}",
                                            tag=f"rec{s}")
                        nc.vector.tensor_copy(r16[:], r32[:])
                        rec16[s] = r16
                    return emit

                def make_bc(s):
                    def emit():
                        bc = sT_ps.tile([P, 256], f32, name=f"bc{s}",
                                        tag="st")
                        nc.tensor.matmul(bc[:], ones_row, rec16[s][:],
                                         start=True, stop=True)
                        nc.vector.tensor_copy(
                            bcsF[:, 256 * s:256 * (s + 1)], bc[:])
                    return emit

                pend = []
                for s in (3, 2, 1, 0):
                    q0 = s * 256
                    cps = [ctx_ps.tile([P, 512], f32, name=f"cps{s}_{i}",
                                       tag="cps") for i in range(4)]
                    dacc = dacc_pool.tile([P, 256], f32, name=f"dacc{s}",
                                          tag="dacc")

                    def st_chunk(c):
                        st = sT_ps.tile([P, 256], f32, name="st", tag="st")
                        for d in range(ND):
                            xc = xt_at(d, 128 * c)
                            nc.tensor.matmul(
                                st[:],
                                xTt[:, xc:xc + 128],
                                GQ[:, 1024 * d + q0:1024 * d + q0 + 256],
                                start=(d == 0), stop=(d == ND - 1))
                        et = exp_pool.tile([P, 256], f16, name="et", tag="et")
                        nc.scalar.activation(et[:], st[:], EXP, scale=1.0 / 32.0)
                        if c >= L[s] - 4:
                            m = 4 * s + (c - (L[s] - 4))
                            et2 = exp_pool.tile([P, 256], f16, name="et2",
                                                tag="et2")
                            nc.vector.tensor_mul(
                                et2[:], et[:], maskT[:, m * 256:(m + 1) * 256])
                            et = et2
                        return et

                    def av_chunk(c, et):
                        if c == 0:
                            nc.vector.tensor_copy(dacc[:], et[:])
                        else:
                            nc.vector.tensor_add(dacc[:], dacc[:], et[:])
                        for d in range(ND):
                            acc = cps[d // 2][:, (d % 2) * 256:
                                              (d % 2) * 256 + 256]
                            # c==0, even half: start=True clears the whole
                            # bank's has_written bits and writes this half;
                            # the odd half's start=False right after then
                            # plain-overwrites -- no explicit zeroing needed.
                            nc.tensor.matmul(
                                acc,
                                xn16[:, 1024 * c + 128 * d:1024 * c + 128 * d + 128],
                                et[:],
                                start=(c == 0 and d % 2 == 0),
                                stop=(c == L[s] - 1),
                                skip_group_check=True)

                    # software pipeline: score chains run 2 chunks ahead of
                    # the AV matmuls.
                    ets = {0: st_chunk(0)}
                    if L[s] > 1:
                        ets[1] = st_chunk(1)
                    for c in range(L[s]):
                        if c in (2, 5) and pend:
                            pend.pop(0)()
                        if c + 2 < L[s]:
                            ets[c + 2] = st_chunk(c + 2)
                        av_chunk(c, ets.pop(c))

                    # evacuate ctx accumulators into the flat per-d tiles,
                    # split DVE/ACT so neither queue backs up.
                    for d in range(ND):
                        dst = ctxC[d][:, 256 * s:256 * (s + 1)]
                        srcp = cps[d // 2][:, (d % 2) * 256:(d % 2) * 256 + 256]
                        if d % 2 == 1:
                            nc.scalar.copy(dst, srcp)
                        else:
                            nc.vector.tensor_copy(dst, srcp)
                    pend.append(make_dsum(s, dacc))
                    pend.append(make_bc(s))

                # ------- phase 3: out^T = (Wv ctx^T) * (1/denominator) ----
                # 512-wide chains: two slots per matmul chain, po ring in
                # the (now free) ctx_ps pool, ob-muls split DVE/gpsimd.
                for hh, qlo in ((1, 512), (0, 0)):
                    for o in range(NO):
                        po = ctx_ps.tile([P, 512], f32, name="poo", tag="cps")
                        for d in range(ND):
                            nc.tensor.matmul(
                                po[:],
                                wvs[:, 1024 * d + 128 * o:1024 * d + 128 * o + 128],
                                ctxC[d][:, qlo:qlo + 512],
                                start=(d == 0), stop=(d == ND - 1))
                        ob = out_pool.tile([P, 512], f32, name="ob", tag="ob")
                        nc.vector.tensor_mul(ob[:], po[:],
                                             bcsF[:, qlo:qlo + 512])
                        nc.sync.dma_start(
                            outT_d[o * P:(o + 1) * P, qlo:qlo + 512],
                            ob[:])
                        if pend and (o % 2 == 1):
                            pend.pop(0)()

    nc.compile()
    return nc


def _get_nc():
    if "nc" not in _NC_CACHE:
        _NC_CACHE["nc"] = _build_nc()
    return _NC_CACHE["nc"]


def _make_masks(h):
    """[128, 16*256] mask tiles: 1.0 where (permuted) key <= query."""
    mk = np.zeros((P, 16 * 256), dtype=np.float16)
    p = np.arange(P)[:, None]
    j = np.arange(256)[None, :]
    for s in range(4):
        g = G[h][s]
        for m in range(4):
            c = L[s] - 4 + m
            gk = PERM[h][c // 2]
            key = 256 * gk + 128 * (c % 2) + p
            mk[:, (4 * s + m) * 256:(4 * s + m + 1) * 256] = (
                key <= (256 * g + j)).astype(np.float16)
    return mk


def kernel(x, W_q, W_k, W_v):
    from concourse.bass_utils import run_bass_kernel_spmd

    x = np.asarray(x, dtype=np.float32)
    x16 = x.astype(np.float16)
    # wqT packed [quarter, p, e, oc] -> [128q + p, 256e + oc]
    wqT16 = np.ascontiguousarray(
        np.asarray(W_q, dtype=np.float32).T.astype(np.float16)
        .reshape(8, P, 4, 256).transpose(2, 1, 0, 3).reshape(4 * P, 8 * 256))
    wk16 = np.ascontiguousarray(np.asarray(W_k, dtype=np.float32)
                                .astype(np.float16))
    wvT = np.ascontiguousarray(np.asarray(W_v, dtype=np.float32).T
                               .astype(np.float16))

    masks_h = [_make_masks(0), _make_masks(1)]
    perm_cols = [np.concatenate([np.arange(256 * g, 256 * (g + 1))
                                 for g in PERM[h]]) for h in range(2)]

    in_maps = []
    for b in range(B):
        # xT packed [blk, p, e, c] -> [128*blk + p, 256*e + c], blk permuted
        xTr = x16[b].T.reshape(8, P, 8, 256)  # [e, p, blk, c]
        for h in range(2):
            xTp = np.ascontiguousarray(
                xTr[:, :, PERM[h], :].transpose(2, 1, 0, 3)
                .reshape(8 * P, 8 * 256))
            in_maps.append(dict(
                xT=xTp,
                xn=np.ascontiguousarray(x16[b][perm_cols[h], :]),
                wqT=wqT16, wk=wk16, wvT=wvT,
                masks=masks_h[h],
            ))

    nc = _get_nc()
    res = run_bass_kernel_spmd(nc, in_maps, core_ids=list(range(NCORES)),
                               trace=bool(os.environ.get("KERNEL_TRACE")))
    if os.environ.get("KERNEL_TRACE"):
        _NC_CACHE["last_results"] = res

    out = np.empty((B, S, DOUT), dtype=np.float32)
    for b in range(B):
        for h in range(2):
            oT = res.results[b * 2 + h]["outT"]
            for s2, g in enumerate(G[h]):
                out[b, g * 256:(g + 1) * 256, :] = \
                    oT[:, s2 * 256:(s2 + 1) * 256].T
    return out
